# revision 28
# baseline (speedup 1.0000x reference)
"""ALRDLinear + KIVI(2-bit key) fused kernel for one TRN2 chip (8 NeuronCores).

    y = x @ W_B^T                    [B,S,R]
    yq = kivi_qdq(y)                 per-channel 2-bit quant along token dim,
                                     groups of 128 tokens
    out = yq @ W_A^T + b_A           [B,S,O]

Sharding: tokens (B*S) are split into 8 contiguous shards of 2048 tokens.
Quantization groups (128 tokens) never straddle shard boundaries, so the
kernel needs no collectives. Weights are replicated per core.

Precision: y is computed with a single f32r (tf32-like, 12-bit mantissa)
TensorEngine pass over host-side rne12-pre-rounded x and W_B^T. The resulting
y error (~1.5e-4 rms) shifts a small fraction of KIVI round() decisions; the
measured end-to-end relative error is ~1.3e-2, within the 2e-2 budget.
Host pre-rounding makes the engine's internal f32r rounding an identity, so
device results match the numpy model. MM2 runs in bf16 (smooth error).

Schedule (per 512-token block b of 4):
  - 32 contraction chunks of MM1 (4 f32r matmuls each) interleave with
    bf16 MM2 matmul groups so f32r LDWEIGHTS hides under bf16 streams:
    12 "carried" groups of block b-2 fill chunks 1..12, then 20 "main"
    groups (oc-major) of block b-1 fill chunks 13..31. Block 1 (no
    carried) starts its mains at chunk 9 in a tk-slow zigzag order.
  - Block b-1's quant drips over chunks 0..8: PSUM->SBUF copies (rb0 on
    Vector, rest on Act), per-rb min/max stats, then qdq activations
    group-major so yq becomes eligible group by group.
  - W_A loads are deferred to block 1's Pool queue, ordered so oc-major
    mains consume o-quarters just in time (block 0 is DMA-bound).
  - Every MM2 group drains PSUM to a small staging tile (Act/Vector
    alternating) and DMAs its [128,512] output slice directly; no
    per-tile assembly buffer.
  - Tail: the last block holds back 14 groups, emitted BEFORE the tail
    quant so they execute during it; tail quant reads PSUM directly and
    computes stats per 2-group half so the first yq lands early.

The gpsimd (Pool) engine is used only for W_A DMA issues: its ucode ops
have us-scale fixed costs and it cannot access PSUM.
"""
import numpy as np
from contextlib import ExitStack

import concourse.bass as bass
import concourse.tile as tile
from concourse import bacc, mybir
from concourse.alu_op_type import AluOpType
from concourse.bass_utils import run_bass_kernel_spmd

F32 = mybir.dt.float32
F32R = mybir.dt.float32r
BF16 = mybir.dt.bfloat16
F16 = mybir.dt.float16
FP8 = mybir.dt.float8e4
MAGIC = float(np.float32(2.0 ** 23))
AF = mybir.ActivationFunctionType
AX = mybir.AxisListType.X

N_CORES = 8
B, S, D, R, O = 4, 4096, 4096, 512, 4096
TOK = B * S // N_CORES


def _build_nc(TOK=TOK, D=D, R=R, O=O, BLK=512, GRP=128,
              xt_bufs=8, yq_bufs=8, psum_y_bufs=6, psum_o_bufs=2,
              out_bufs=3, ysb_bufs=4, carry=8):
    P = 128
    DC = D // P
    RB = R // P
    NB = TOK // BLK
    GPB = BLK // GRP
    OCW = 512
    OC = O // OCW
    TKC = BLK // P
    assert GRP == P

    nc = bacc.Bacc()
    NB_ = TOK // BLK
    PK = BLK + BLK // 2
    xpk = nc.declare_dram_parameter("xpk", [D, NB_, PK], F16, isOutput=False)
    wpk = nc.declare_dram_parameter("wpk", [D, R + R // 2], F16, isOutput=False)
    wat = nc.declare_dram_parameter("wat", [R, O], BF16, isOutput=False)
    out = nc.declare_dram_parameter("out", [TOK, O], BF16, isOutput=True)

    with tile.TileContext(nc) as tc, ExitStack() as ctx:
        pool_w = ctx.enter_context(tc.tile_pool(name="w_persist", bufs=1))
        pool_xt = ctx.enter_context(tc.tile_pool(name="xt", bufs=xt_bufs))
        pool_x3 = ctx.enter_context(tc.tile_pool(name="x3", bufs=6))
        pool_t = ctx.enter_context(tc.tile_pool(name="tq", bufs=4))
        pool_ysb = ctx.enter_context(tc.tile_pool(name="ysb", bufs=ysb_bufs))
        pool_yq = ctx.enter_context(tc.tile_pool(name="yq", bufs=yq_bufs))
        pool_sm = ctx.enter_context(tc.tile_pool(name="small", bufs=6))
        pool_stg = ctx.enter_context(tc.tile_pool(name="outstg", bufs=4))
        pool_py = ctx.enter_context(
            tc.tile_pool(name="psum_y", bufs=psum_y_bufs, space="PSUM"))
        pool_po = ctx.enter_context(
            tc.tile_pool(name="psum_o", bufs=psum_o_bufs, space="PSUM"))

        wr_sb = [None] * DC
        wat_sb = [None] * RB
        # wat slice load order: first the slices every oc group of each rb
        # needs (o-quarter 0 for all rb), then quarter 1, etc.
        WAT_ORDER = [rb * 4 + s for s in range(4) for rb in range(RB)]

        def load_wr_chunk(c):
            w3 = pool_x3.tile([P, R + R // 2], F16, tag="w3", name=f"w3_{c}")
            nc.scalar.dma_start(out=w3, in_=wpk[c * P:(c + 1) * P, :])
            w_t = pool_w.tile([P, R], F32R, tag=f"wr{c}", name=f"wr_{c}")
            nc.vector.scalar_tensor_tensor(w_t, w3[:, R:].bitcast(FP8),
                                           2.0 ** -15, w3[:, :R],
                                           AluOpType.mult, AluOpType.add)
            wr_sb[c] = w_t

        def alloc_mm2_weights():
            for rb in range(RB):
                wat_sb[rb] = pool_w.tile([P, O], BF16, tag=f"wat{rb}",
                                         name=f"wat_{rb}")

        def load_wat_slice(i):
            # i in 0..RB*4-1: load [P, O/4] slice of one wat row-block
            rb, s = divmod(i, 4)
            o0, o1 = s * (O // 4), (s + 1) * (O // 4)
            nc.gpsimd.dma_start(out=wat_sb[rb][:, o0:o1],
                                in_=wat[rb * P:(rb + 1) * P, o0:o1])

        def emit_warmup():
            # ~28 tiny matmuls on zeros while the first DMAs land: keeps the
            # PE HAM activity window busy so real matmuls start at full clock
            w0 = pool_sm.tile([P, P], F32, tag="warm_w")
            nc.vector.memset(w0[:], 0.0)
            ps = pool_py.tile([P, GPB, GRP], F32, tag="py", name="py_warm")
            for i in range(28):
                nc.tensor.matmul(ps[:, 0, :], w0[:], w0[:], start=True,
                                 stop=True)

        def emit_mm2_group(b, yq, oc, tk):
            tok0 = b * BLK
            o0, o1 = oc * OCW, (oc + 1) * OCW
            po = pool_po.tile([P, OCW], F32, tag="po", name=f"po_{b}_{oc}_{tk}")
            for rb in range(RB):
                nc.tensor.matmul(po[:], yq[rb][:, tk, :], wat_sb[rb][:, o0:o1],
                                 start=(rb == 0), stop=(rb == RB - 1))
            stg = pool_stg.tile([P, OCW], BF16, tag="stg",
                                name=f"stg_{b}_{oc}_{tk}")
            if oc % 2 == 0:
                nc.scalar.activation(out=stg, in_=po[:], func=AF.Identity)
            else:
                nc.vector.tensor_copy(out=stg, in_=po[:])
            rows = out[tok0 + tk * P: tok0 + (tk + 1) * P, o0:o1]
            nc.sync.dma_start(out=rows, in_=stg)

        def mm2_group_list(b, yq, oc_major=False):
            if oc_major:
                return [(b, yq, oc, tk) for oc in range(OC) for tk in range(TKC)]
            return [(b, yq, oc, tk) for tk in range(TKC) for oc in range(OC)]

        def emit_quant(b, py):
            """PSUM->SBUF copies on Pool now; returns yq tiles plus a list of
            drip closures (executed at chunks 0..8 of the next block):
            per-rb stats (reading PSUM), then smalls, then per-group acts so
            yq becomes available group-major (token-tile-major)."""
            ysb_l = [pool_ysb.tile([P, GPB, GRP], F32, tag="ysb",
                                   name=f"ysb_{b}_{rb}") for rb in range(RB)]
            # rb0 on Vector, rb1 first on Act: the next block's chunk-0
            # matmuls reuse the PSUM slots of rb0/rb1, so those two copies
            # must land fast and in parallel
            nc.vector.tensor_copy(out=ysb_l[0], in_=py[0][:])
            for rb in (1, 2, 3):
                nc.scalar.activation(out=ysb_l[rb], in_=py[rb][:],
                                     func=AF.Identity)
            yq_l = [pool_yq.tile([P, GPB, GRP], BF16, tag="yq",
                                 name=f"yq_{b}_{rb}") for rb in range(RB)]
            mn_l = [pool_sm.tile([P, GPB], F32, tag="mn", name=f"mn_{b}_{rb}")
                    for rb in range(RB)]
            mx_l = [pool_sm.tile([P, GPB], F32, tag="mx", name=f"mx_{b}_{rb}")
                    for rb in range(RB)]
            sc_l = [pool_sm.tile([P, GPB], F32, tag="scale", name=f"sc_{b}_{rb}")
                    for rb in range(RB)]
            rs_l = [pool_sm.tile([P, GPB], F32, tag="rscale", name=f"rs_{b}_{rb}")
                    for rb in range(RB)]
            nb_l = [pool_sm.tile([P, GPB], F32, tag="nbias", name=f"nb_{b}_{rb}")
                    for rb in range(RB)]
            t_l = [pool_t.tile([P, GPB, GRP], F32, tag="t", name=f"t_{b}_{rb}")
                   for rb in range(RB)]

            def make_stats(rb):
                def stats():
                    # reads the SBUF copy: py PSUM bufs are reallocated by the
                    # next block's matmuls before these drips execute
                    src = ysb_l[rb]
                    nc.vector.tensor_reduce(mn_l[rb], src, AX, AluOpType.min)
                    nc.vector.tensor_reduce(mx_l[rb], src, AX, AluOpType.max)
                return stats

            def smalls():
                ve = nc.vector
                for rb in range(RB):
                    ve.tensor_tensor(sc_l[rb], mx_l[rb], mn_l[rb],
                                     AluOpType.subtract)
                    ve.tensor_scalar(sc_l[rb], sc_l[rb], 1.0 / 3.0, 1e-8,
                                     AluOpType.mult, AluOpType.max)
                    ve.reciprocal(out=rs_l[rb], in_=sc_l[rb])
                    ve.scalar_tensor_tensor(nb_l[rb], mn_l[rb], -1.0, rs_l[rb],
                                            AluOpType.mult, AluOpType.mult)

            def make_gact(g):
                def gact():
                    for rb in range(RB):
                        nc.scalar.activation(out=t_l[rb][:, g, :],
                                             in_=ysb_l[rb][:, g, :],
                                             func=AF.Identity,
                                             bias=nb_l[rb][:, g:g + 1],
                                             scale=rs_l[rb][:, g:g + 1])
                    for rb in range(RB):
                        nc.vector.tensor_scalar(t_l[rb][:, g, :],
                                                t_l[rb][:, g, :],
                                                MAGIC, MAGIC,
                                                AluOpType.add,
                                                AluOpType.subtract)
                    for rb in range(RB):
                        nc.scalar.activation(out=yq_l[rb][:, g, :],
                                             in_=t_l[rb][:, g, :],
                                             func=AF.Identity,
                                             bias=mn_l[rb][:, g:g + 1],
                                             scale=sc_l[rb][:, g:g + 1])
                return gact

            drips = ([make_stats(rb) for rb in range(RB)] + [smalls] +
                     [make_gact(g) for g in range(GPB)])
            return yq_l, drips

        def emit_quant_tail(b, py):
            """Last block: stats and acts straight from PSUM, computed per
            2-group half so the first token tiles' yq land a few us after
            mm1 ends and the tail MM2 starts early."""
            yq_l = [pool_yq.tile([P, GPB, GRP], BF16, tag="yq",
                                 name=f"yqt_{rb}") for rb in range(RB)]
            t_l = [pool_t.tile([P, GPB, GRP], F32, tag="t", name=f"tt_{rb}")
                   for rb in range(RB)]
            H = 2  # groups per half
            for h in range(GPB // H):
                g0, g1 = h * H, (h + 1) * H
                mn_l, mx_l, sc_l, rs_l, nb_l = [], [], [], [], []
                ve = nc.vector
                for rb in range(RB):
                    mn = pool_sm.tile([P, H], F32, tag="mn",
                                      name=f"mnt_{h}_{rb}")
                    mx = pool_sm.tile([P, H], F32, tag="mx",
                                      name=f"mxt_{h}_{rb}")
                    nc.vector.tensor_reduce(mn, py[rb][:, g0:g1, :], AX,
                                            AluOpType.min)
                    nc.vector.tensor_reduce(mx, py[rb][:, g0:g1, :], AX,
                                            AluOpType.max)
                    sc = pool_sm.tile([P, H], F32, tag="scale",
                                      name=f"sct_{h}_{rb}")
                    rs = pool_sm.tile([P, H], F32, tag="rscale",
                                      name=f"rst_{h}_{rb}")
                    nb = pool_sm.tile([P, H], F32, tag="nbias",
                                      name=f"nbt_{h}_{rb}")
                    ve.tensor_tensor(sc, mx, mn, AluOpType.subtract)
                    ve.tensor_scalar(sc, sc, 1.0 / 3.0, 1e-8,
                                     AluOpType.mult, AluOpType.max)
                    ve.reciprocal(out=rs, in_=sc)
                    ve.scalar_tensor_tensor(nb, mn, -1.0, rs,
                                            AluOpType.mult, AluOpType.mult)
                    mn_l.append(mn); mx_l.append(mx)
                    sc_l.append(sc); rs_l.append(rs); nb_l.append(nb)
                for g in range(g0, g1):
                    gh = g - g0
                    for rb in range(RB):
                        nc.scalar.activation(out=t_l[rb][:, g, :],
                                             in_=py[rb][:, g, :],
                                             func=AF.Identity,
                                             bias=nb_l[rb][:, gh:gh + 1],
                                             scale=rs_l[rb][:, gh:gh + 1])
                    for rb in range(RB):
                        nc.vector.tensor_scalar(t_l[rb][:, g, :],
                                                t_l[rb][:, g, :],
                                                MAGIC, MAGIC,
                                                AluOpType.add,
                                                AluOpType.subtract)
                    for rb in range(RB):
                        nc.scalar.activation(out=yq_l[rb][:, g, :],
                                             in_=t_l[rb][:, g, :],
                                             func=AF.Identity,
                                             bias=mn_l[rb][:, gh:gh + 1],
                                             scale=sc_l[rb][:, gh:gh + 1])
            return yq_l

        def emit_mm1(b, carried, mains, drips):
            """One block of MM1, with carried groups (block b-2) filling
            chunks 1..8, quant drips (block b-1) at chunks 0..8, and main
            groups (block b-1) from chunk 9 on."""
            mains = list(mains)
            carried = list(carried)
            extra_at = {16}
            py = [pool_py.tile([P, GPB, GRP], F32, tag="py", name=f"py_{b}_{rb}")
                  for rb in range(RB)]
            for c in range(DC):
                if b == 0:
                    load_wr_chunk(c)
                x3 = pool_x3.tile([P, PK], F16, tag="x3")
                nc.sync.dma_start(out=x3, in_=xpk[c * P:(c + 1) * P, b, :])
                x_in = pool_xt.tile([P, BLK], F32R, tag="x_in")
                nc.vector.scalar_tensor_tensor(x_in, x3[:, BLK:].bitcast(FP8),
                                               2.0 ** -15, x3[:, :BLK],
                                               AluOpType.mult, AluOpType.add)
                if b == 1 and c % 2 == 0:
                    # wat slices ride the Pool queue after the recon emission;
                    # oc-major mains consume o-quarters progressively so this
                    # just-in-time order works bandwidth-wise
                    load_wat_slice(WAT_ORDER[c // 2])
                first = c == 0
                last = c == DC - 1
                for rb in range(RB):
                    w0, w1 = rb * P, (rb + 1) * P
                    nc.tensor.matmul(py[rb][:], wr_sb[c][:, w0:w1], x_in,
                                     start=first, stop=last)
                if c <= 8 and drips:
                    drips.pop(0)()
                if 1 <= c <= 12 and carried:
                    emit_mm2_group(*carried.pop(0))
                main_from = 13 if b != 1 else 9
                if c >= main_from and mains:
                    emit_mm2_group(*mains.pop(0))
                    if c in extra_at and mains:
                        emit_mm2_group(*mains.pop(0))
            # anything left (shouldn't happen in steady state)
            for g in carried:
                emit_mm2_group(*g)
            for g in mains:
                emit_mm2_group(*g)
            return py

        emit_warmup()
        alloc_mm2_weights()
        # 12 carried groups fill chunks 1..12 (hide f32r LDWEIGHTS entries and
        # cover the tail-quant latency); 20 mains at c=13..31 (+extra at 16)
        n_main = 20
        prev_yq = None
        drips = []
        carried = []
        for b in range(NB):
            lst = (mm2_group_list(b - 1, prev_yq, oc_major=True)
                   if prev_yq is not None else [])
            if b == 1:
                # block 1 has no carried groups, so mains start at c=9 and
                # must not demand late quant groups early: zigzag oc pairs
                # so tk (= quant group) advances slowly
                lst = [(b - 1, prev_yq, ob + d, tk)
                       for ob in (0, 2, 4, 6) for tk in range(TKC)
                       for d in (0, 1)]
            nm = n_main if b != NB - 1 else 18  # hold 14 for the tail quant
            mains, carry_next = lst[:nm], lst[nm:]
            py = emit_mm1(b, carried, mains, drips)
            carried = carry_next
            if b == NB - 1:
                for g in carried:  # held groups run during the tail quant
                    emit_mm2_group(*g)
                yq_tail = emit_quant_tail(b, py)
                for g in mm2_group_list(b, yq_tail):  # tk-major: yq arrives
                    emit_mm2_group(*g)                # group-major
            else:
                prev_yq, drips = emit_quant(b, py)
    nc.finalize()
    return nc


def _pack_hi_lo(a):
    """Pack f32 [D, N] as fp16(a) followed by fp8((a - fp16)*2^15) bytes,
    in one fp16-typed [D, N + N//2] array (single contiguous DMA line)."""
    import ml_dtypes
    Dd, N = a.shape
    hi = a.astype(np.float16)
    resid = (a - hi.astype(np.float32)) * np.float32(2.0 ** 15)
    lo8 = np.asarray(resid, dtype=ml_dtypes.float8_e4m3fn)
    pack = np.empty((Dd, N + N // 2), dtype=np.float16)
    pu8 = pack.view(np.uint8)
    pu8[:, :2 * N] = hi.view(np.uint8)
    pu8[:, 2 * N:] = lo8.view(np.uint8)
    return pack


def _make_in_maps(input, W_B, W_A, b_A, BLK=512):
    import ml_dtypes
    x = np.ascontiguousarray(np.asarray(input, dtype=np.float32))
    W_B = np.asarray(W_B, dtype=np.float32)
    W_A = np.asarray(W_A, dtype=np.float32)
    b_A = np.asarray(b_A, dtype=np.float32)
    Bi, Si, Di = x.shape

    toks = Bi * Si
    tok_pc = toks // N_CORES
    xf = np.ascontiguousarray(x.reshape(toks, Di))
    wbt = np.ascontiguousarray(W_B.T).astype(np.float32)
    wpk = _pack_hi_lo(wbt)
    wat = np.ascontiguousarray(W_A.T).astype(ml_dtypes.bfloat16)
    NB_ = tok_pc // BLK
    PK = BLK + BLK // 2
    in_maps = []
    for c in range(N_CORES):
        shard = np.ascontiguousarray(xf[c * tok_pc:(c + 1) * tok_pc].T)
        xpk = np.empty((Di, NB_, PK), dtype=np.float16)
        xu8 = xpk.view(np.uint8)
        for b in range(NB_):
            blk = _pack_hi_lo(shard[:, b * BLK:(b + 1) * BLK])
            xu8[:, b, :] = blk.view(np.uint8)
        in_maps.append({"xpk": xpk, "wpk": wpk, "wat": wat})
    return in_maps, (Bi, Si, Di, W_B.shape[0], W_A.shape[0], tok_pc)


def kernel(input, W_B, W_A, b_A):
    in_maps, (Bi, Si, Di, Ri, Oi, tok_pc) = _make_in_maps(input, W_B, W_A, b_A)
    nc = _build_nc(TOK=tok_pc, D=Di, R=Ri, O=Oi)
    res = run_bass_kernel_spmd(nc, in_maps, core_ids=list(range(N_CORES)),
                               trace=False)
    b_A = np.asarray(b_A, dtype=np.float32)
    out = np.concatenate([np.asarray(res.results[c]["out"]).astype(np.float32)
                          for c in range(N_CORES)], axis=0)
    out += b_A
    return out.reshape(Bi, Si, Oi)
